# revision 54
# baseline (speedup 1.0000x reference)
"""Multi-head attention (B=2, S=2048, D=768, H=12) on 8 TRN2 NeuronCores.

Sharding: data-parallel over batch x tensor-parallel over heads.
  core c -> batch c//4, heads 3*(c%4) .. 3*(c%4)+2
Each core computes its 3 heads end-to-end plus the partial output
projection (its 192 rows of w_proj), emitted TRANSPOSED as out^T [D, S].
Host sums the 4 partials per batch, transposes, and adds b_proj plus the
folded V-bias term (bv @ w_proj), so the device never touches bv.

Performance structure (per core):
  The 128x128 PE array is 16 independent 32x32 sub-arrays; matmuls whose
  (row_grp, col_grp) strips are disjoint run CONCURRENTLY.  tile_position
  auto-derives as (lhsT.base_partition, out.base_partition).
    - scores have K=64: a pair of units whose Q/K live at SBUF bases (0, 64)
      co-runs 2x.  h0 lives at base 0, h1 at base 64, and h2 is produced at
      BOTH bases so score pairs always split rows.
    - AV has K=128: split into K=64 halves (v1/es partition halves); the lo
      half of one unit co-runs with the hi half of the other.
  Steady state runs one 512-column key-chunk per round: scores pair (216ns)
  + 4 AV half-MMs (432ns) + interleaved proj/broadcast work.  PSUM budget
  (16KB/partition = 8 banks): scores tag "s" 3x1 bank (so scores(g+1) never
  waits on exp(g-1) -- the psum-recycle recurrence that serialized the old
  kernel), po tag "o" 2x2 banks, aux tag "x" 1x1 bank (proj psum + the
  1->64 reciprocal broadcast).
  exp alternates per round between ACT (table exp) and DVE (Schraudolph:
  scores*128/ln2 + bias -> int16 == bf16 bits of ~exp; ~1.8% rms on half
  the groups), keeping each engine under the PE round time.
  Normalize is split: po psum reads (DVE) are emitted at the pair boundary
  so the po slots recycle immediately; the reciprocal row is broadcast
  1->64 rows with a warm PE matmul and the final multiply is a single
  scalar_tensor_tensor.  The PE never idles long enough for the HAM clock
  gate to re-throttle it to 1.2GHz (which cost the old kernel ~96us of
  half-clock time).
"""

from contextlib import ExitStack

import numpy as np

import concourse.bass as bass
import concourse.mybir as mybir
import concourse.tile as tile
from concourse import bacc
from concourse.bass_utils import run_bass_kernel_spmd

B, S, D = 2, 2048, 768
H, HD = 12, 64
SCALE = HD**-0.5
NCORES = 8
HPC = 3  # heads per core
P = 128
KD = D // P  # 6 chunks of contraction dim for projections
QB = 512  # query block (free dim of score matmuls)
NQB = S // QB  # 4
NKC = S // P  # 16 key chunks == rounds per pair
F32 = mybir.dt.float32
BF16 = mybir.dt.bfloat16
I16 = mybir.dt.int16
EXP = mybir.ActivationFunctionType.Exp
MULT = mybir.AluOpType.mult
ADD = mybir.AluOpType.add

# Schraudolph exp-in-bf16-bits: i16 = round(s*A16 + B16) are the bf16 bits
# of ~exp(s).  B16 = 127*128 - 7.5 centers the piecewise-linear error.
A16 = 128.0 / np.log(2.0)
B16 = 16256.0 - 7.5

_CACHE = {}


def _build_nc():
    nc = bacc.Bacc("TRN2", target_bir_lowering=False, debug=False)
    # x / weight tensors arrive as uint16 = bf16 bit patterns (host-side
    # round-to-nearest conversion) — half the HBM stream of f32.
    U16 = mybir.dt.uint16
    xT_d = nc.dram_tensor("xT", [D, S], U16, kind="ExternalInput").ap()
    wqk_d = nc.dram_tensor("wqk", [D, 2 * HPC * HD], U16, kind="ExternalInput").ap()
    bqk_d = nc.dram_tensor("bqk", [P, 4], F32, kind="ExternalInput").ap()
    wv_d = nc.dram_tensor("wv", [D, HPC * HD], U16, kind="ExternalInput").ap()
    wp_d = nc.dram_tensor("wp", [HPC * HD, D], U16, kind="ExternalInput").ap()
    # uint16 = bf16 bit patterns (the axon PJRT bridge can't move bf16)
    out_d = nc.dram_tensor("out", [D, S], mybir.dt.uint16, kind="ExternalOutput").ap()

    with tile.TileContext(nc) as tc, ExitStack() as ctx:
        const = ctx.enter_context(tc.tile_pool(name="const", bufs=1))
        stage = ctx.enter_context(tc.tile_pool(name="stage", bufs=2))
        es_pool = ctx.enter_context(tc.tile_pool(name="es", bufs=6))
        ot_pool = ctx.enter_context(tc.tile_pool(name="ot", bufs=2))
        rc_pool = ctx.enter_context(tc.tile_pool(name="rc", bufs=2))
        out_pool = ctx.enter_context(tc.tile_pool(name="outsb", bufs=3))

        # ---- load inputs ----
        # Direct DMA into the const tiles (uint16 bitcast = bf16 bits), in
        # consumption order: xt chunk 0 first so the QKV matmuls start ~5us
        # in and pipeline behind the rest of the x stream.
        bqk_sb = const.tile([P, 4], F32, tag="bqk")
        nc.gpsimd.dma_start(out=bqk_sb[:], in_=bqk_d[:, :])
        # warm the ACT exp table while DMAs stream
        warm = const.tile([1, 2], BF16, tag="warm")
        nc.vector.memset(warm[:], 0.0)
        nc.scalar.activation(warm[:], warm[:], EXP)
        ones_bf = const.tile([HD + 1, HD], BF16, tag="ones")
        nc.vector.memset(ones_bf[:], 1.0)

        # Matmul-input tiles are staged DMA -> stage tile -> DVE/ACT copy, so
        # every matmul's input-ready dependency is a cheap engine semaphore
        # (waiting on raw DMA semaphores measured ~+60ns on EVERY matmul).
        xt = [const.tile([P, S], BF16, tag=f"xt{i}", name=f"xt{i}") for i in range(KD)]
        wqk_sb = [
            const.tile([P, 2 * HPC * HD], BF16, tag=f"wqk{i}", name=f"wqk{i}")
            for i in range(KD)
        ]
        wv_sb = [
            const.tile([P, HPC * HD], BF16, tag=f"wv{i}", name=f"wv{i}")
            for i in range(KD)
        ]
        U16 = mybir.dt.uint16

        def load(dma_eng, dst, src_ap, kind, eng):
            st_t = stage.tile(list(dst.shape), BF16, tag=f"stg_{kind}")
            dma_eng.dma_start(out=st_t[:].bitcast(U16), in_=src_ap)
            if eng == "act":
                nc.scalar.copy(dst[:], st_t[:])
            else:
                nc.vector.tensor_copy(dst[:], st_t[:])

        # x loads are split per (chunk, 512-col block), first QKV bank's six
        # pieces first: bank nb computes while bank nb+1's pieces stream in.
        def load_xt_piece(i, nb):
            cols = slice(nb * QB, (nb + 1) * QB)
            st_t = stage.tile([P, QB], BF16, tag="stg_xt", bufs=8)
            (nc.sync if (i + nb) % 2 == 0 else nc.gpsimd).dma_start(
                out=st_t[:].bitcast(U16), in_=xT_d[i * P : (i + 1) * P, cols]
            )
            if (i + nb) % 2:
                nc.scalar.copy(xt[i][:, cols], st_t[:])
            else:
                nc.vector.tensor_copy(xt[i][:, cols], st_t[:])

        for i in range(KD):
            load_xt_piece(i, 0)
            if i < 3:
                load(nc.gpsimd, wqk_sb[i], wqk_d[i * P : (i + 1) * P, :], "wqk", "act")
        for i in range(3, KD):
            load(nc.gpsimd, wqk_sb[i], wqk_d[i * P : (i + 1) * P, :], "wqk", "act")
        for nb in range(1, NQB):
            for i in range(KD):
                load_xt_piece(i, nb)
        for i in range(KD):
            load(nc.sync, wv_sb[i], wv_d[i * P : (i + 1) * P, :], "wv", "dve")
        # output projection weights: h0+h1 stacked on all 128 partitions so
        # each proj chunk is one K=128 matmul plus a K=64 for h2
        wp01 = const.tile([P, D], BF16, tag="wp01")
        load(nc.gpsimd, wp01, wp_d[0 : 2 * HD, :], "wp01", "act")
        wp2 = const.tile([HD, D], BF16, tag="wp2")
        load(nc.gpsimd, wp2, wp_d[2 * HD : 3 * HD, :], "wp2", "act")
        # base-0 copy of h1's rows: the LAST qb projects unstacked (3 K=64
        # matmuls) so its chunks never wait on the ot01 DMA partition-shift
        wp1 = const.tile([HD, D], BF16, tag="wp1")
        load(nc.gpsimd, wp1, wp_d[HD : 2 * HD, :], "wp1", "act")

        # v1 tiles ([v_h | ones] per head) memset early, during the loads
        v1 = []
        for st in range(NKC):
            t = const.tile([P, HPC * (HD + 1)], BF16, tag=f"v1_{st}")
            nc.vector.memset(t[:], 1.0)
            v1.append(t)

        # ---- Q^T / K^T projection, k-outer in 2 passes of 8 psum banks ----
        # h2's Q/K are produced at BOTH partition bases so score pairs can
        # always put one lhsT at rows 0-63 and the other at rows 64-127.
        tq0 = const.tile([P, S], BF16, tag="tq0")  # [q_h0; q_h1]
        tk0 = const.tile([P, S], BF16, tag="tk0")  # [k_h0; k_h1]
        tq1 = const.tile([HD, S], BF16, tag="tq1")  # q_h2 @ base 0
        tk1 = const.tile([HD, S], BF16, tag="tk1")  # k_h2 @ base 0
        tq1h = const.tile([P, S], BF16, tag="tq1h")  # rows 64:128 = q_h2
        tk1h = const.tile([P, S], BF16, tag="tk1h")  # rows 64:128 = k_h2
        CQ2, CK2 = P, 2 * P + HD  # wqk col starts of q2 / k2
        # pass -> (full-m spec, colpair spec): colpair writes lo/hi halves of
        # one psum bank concurrently (auto tile_position (0,0) / (0,64)).
        passes = [
            ((tq0, 0, 0), (tq1, CQ2, 1, tk1h, CK2, 3)),
            ((tk0, P + HD, 2), (tk1, CK2, 3, tq1h, CQ2, 1)),
        ]
        qk_ctx = ExitStack()
        ps_qk = qk_ctx.enter_context(tc.tile_pool(name="ps_qk", bufs=8, space="PSUM"))
        for (dst0, c00, b0), (dlo, clo, blo, dhi, chi, bhi) in passes:
            # bank-outer / k-inner: each nb's psum banks finish early so the
            # bias-add drains overlap the next bank's matmuls instead of
            # piling up (PE-idle) at the end of the pass
            for nb in range(NQB):
                psf = ps_qk.tile([P, QB], F32, tag="qk", name=f"psf{b0}_{nb}")
                psc = ps_qk.tile([P, QB], F32, tag="qk", name=f"psc{b0}_{nb}")
                for k in range(KD):
                    x_ap = xt[k][:, nb * QB : (nb + 1) * QB]
                    st = dict(start=(k == 0), stop=(k == KD - 1))
                    nc.tensor.matmul(psf[:], wqk_sb[k][:, c00 : c00 + P], x_ap, **st)
                    nc.tensor.matmul(psc[0:HD], wqk_sb[k][:, clo : clo + HD], x_ap, **st)
                    nc.tensor.matmul(psc[HD:P], wqk_sb[k][:, chi : chi + HD], x_ap, **st)
                cols = slice(nb * QB, (nb + 1) * QB)
                # bias-adds drain the psum banks, balanced across ACT and DVE
                if nb % 2:
                    nc.vector.tensor_scalar_add(dst0[:, cols], psf[:], bqk_sb[:, b0 : b0 + 1])
                    nc.vector.tensor_scalar_add(dlo[:, cols], psc[0:HD], bqk_sb[0:HD, blo : blo + 1])
                    nc.scalar.activation(dhi[HD:P, cols], psc[HD:P],
                                         mybir.ActivationFunctionType.Identity,
                                         bias=bqk_sb[HD:P, bhi : bhi + 1])
                else:
                    nc.scalar.activation(dst0[:, cols], psf[:],
                                         mybir.ActivationFunctionType.Identity,
                                         bias=bqk_sb[:, b0 : b0 + 1])
                    nc.scalar.activation(dlo[:, cols], psc[0:HD],
                                         mybir.ActivationFunctionType.Identity,
                                         bias=bqk_sb[0:HD, blo : blo + 1])
                    nc.vector.tensor_scalar_add(dhi[HD:P, cols], psc[HD:P], bqk_sb[HD:P, bhi : bhi + 1])
        qk_ctx.close()

        # ---- V projection, k-outer in 2 passes of 8 key chunks ----
        v_ctx = ExitStack()
        ps_v = v_ctx.enter_context(tc.tile_pool(name="ps_v", bufs=8, space="PSUM"))
        for pas in (range(0, 8), range(8, NKC)):
            psl = [ps_v.tile([P, HPC * HD], F32, tag="v", name=f"pv{st}") for st in pas]
            # chunk-outer so each psum bank finishes (and its drain + the
            # next tenant's matmuls start) without waiting for the whole pass
            for st, pv in zip(pas, psl):
                for k in range(KD):
                    nc.tensor.matmul(
                        pv[:],
                        xt[k][:, st * P : (st + 1) * P],
                        wv_sb[k][:],
                        start=(k == 0),
                        stop=(k == KD - 1),
                    )
                # drain psum -> v1 head blocks, alternating DVE/ACT per chunk
                for h in range(HPC):
                    dst = v1[st][:, h * (HD + 1) : h * (HD + 1) + HD]
                    src = pv[:, h * HD : (h + 1) * HD]
                    if st % 2:
                        nc.scalar.copy(dst, src)
                    else:
                        nc.vector.tensor_copy(dst, src)
        v_ctx.close()

        # ---- attention steady state ----
        attn_ctx = ExitStack()
        ps_s = attn_ctx.enter_context(tc.tile_pool(name="ps_s", bufs=3, space="PSUM"))
        ps_o = attn_ctx.enter_context(tc.tile_pool(name="ps_o", bufs=4, space="PSUM"))
        ps_x = attn_ctx.enter_context(tc.tile_pool(name="ps_x", bufs=1, space="PSUM"))

        def qk_loc(h, base64):
            # -> (q tile, k tile, partition offset)
            if h == 0:
                return tq0, tk0, 0
            if h == 1:
                return tq0, tk0, HD
            return (tq1h, tk1h, HD) if base64 else (tq1, tk1, 0)

        units = [(qb, h) for qb in range(NQB) for h in range(HPC)]

        def emit_scores(pair, bases, g):
            # one key chunk per round: 2 score MMs co-run (rows split via the
            # 0/64 lhsT bases), then one exp per engine (ACT / DVE alternate
            # per round so neither engine saturates).
            ess = []
            for i, ((qb, h), b64) in enumerate(zip(pair, bases)):
                qt, kt, o = qk_loc(h, b64)
                ps = ps_s.tile([P, QB], F32, tag="s", name=f"ps{qb}_{h}_{g}")
                nc.tensor.matmul(
                    ps[:],
                    kt[o : o + HD, g * P : (g + 1) * P],
                    qt[o : o + HD, qb * QB : (qb + 1) * QB],
                    start=True,
                    stop=True,
                )
                es = es_pool.tile([P, QB], BF16, tag="es", name=f"es{qb}_{h}_{g}")
                if (g + i) % 2:
                    nc.vector.tensor_scalar(
                        out=es[:].bitcast(I16), in0=ps[:], scalar1=A16, scalar2=B16,
                        op0=MULT, op1=ADD,
                    )
                else:
                    nc.scalar.activation(es[:], ps[:], EXP)
                ess.append(es)
            return ess

        def emit_av(pair, pos, ess, kc):
            # one K=128 matmul per unit into a single-region po [65, 512]:
            # row 64 (the v1 ones column) accumulates the softmax denominator
            # in place, so normalize needs no halves-add.
            for (qb, h), po, es in zip(pair, pos, ess):
                nc.tensor.matmul(
                    po[:],
                    v1[kc][:, h * (HD + 1) : (h + 1) * (HD + 1)],
                    es[:],
                    start=(kc == 0),
                    stop=(kc == NKC - 1),
                )

        def normalize_a(u, po, i):
            # All the po (psum) readers, emitted at the pair boundary so the
            # po slot recycles before the next pair's AV needs it.  One
            # [1,512] bf16-converting denom-row copy + one [64,512] copy,
            # split across ACT/DVE (alternating by unit).
            dsb2h = rc_pool.tile([HD + 1, QB], BF16, tag="rc2")
            ocp = rc_pool.tile([HD, QB], F32, tag="ocp")
            if i % 2:
                nc.scalar.copy(dsb2h[HD : HD + 1, :], po[HD : HD + 1, :])
                nc.vector.tensor_copy(ocp[:], po[0:HD, :])
            else:
                nc.vector.tensor_copy(dsb2h[HD : HD + 1, :], po[HD : HD + 1, :])
                nc.scalar.copy(ocp[:], po[0:HD, :])
            return dsb2h, ocp

        ot01_map = {}  # qb -> stacked [h0; h1] ot tile

        def normalize_b(u, dsb2h, ocp):
            # broadcast the denom from row 64 to rows 0-63 on the PE (bf16
            # matmul — stays in the matmul stream, keeps HAM warm);
            # reciprocal must run at base partition 0 (custom DVE ops are
            # silent no-ops elsewhere).  h0/h1 outputs land stacked on
            # partitions 0-63 / 64-127 (h1 via an SBUF->SBUF DMA partition
            # shift, off the critical path) so each projection chunk is one
            # K=128 matmul + one K=64 instead of three K=64.
            qb, h = u
            prb = ps_x.tile([HD, QB], F32, tag="x", name=f"prb{qb}_{h}")
            nc.tensor.matmul(
                prb[:], ones_bf[HD : HD + 1, :], dsb2h[HD : HD + 1, :],
                start=True, stop=True,
            )
            rb = rc_pool.tile([HD, QB], F32, tag="rb")
            nc.vector.reciprocal_approx_fast(rb[:], prb[:])
            if h == 0:
                ot01 = ot_pool.tile([P, QB], BF16, tag="ot01", name=f"ot01_{qb}")
                ot01_map[qb] = [ot01]
                nc.gpsimd.tensor_tensor(out=ot01[0:HD, :], in0=ocp[:], in1=rb[:], op=MULT)
            elif h == 1:
                tmp = ot_pool.tile([HD, QB], BF16, tag="otmp", name=f"otmp_{qb}")
                nc.gpsimd.tensor_tensor(out=tmp[:], in0=ocp[:], in1=rb[:], op=MULT)
                nc.sync.dma_start(out=ot01_map[qb][0][HD:P, :], in_=tmp[:])
            else:
                ot2 = ot_pool.tile([HD, QB], BF16, tag="ot2", name=f"ot2_{qb}")
                nc.gpsimd.tensor_tensor(out=ot2[:], in0=ocp[:], in1=rb[:], op=MULT)
                pending_proj.append((qb, (*ot01_map.pop(qb), ot2), 0))

        pending_proj = []  # (qb, [ot_h0, ot_h1, ot_h2], next_dchunk)
        proj_state = [None]  # (qb, ots, dc, pp, next_h) mid-chunk
        tail_rot = [0]

        def emit_proj_step(tail=False):
            # one projection matmul per call; the psum drain (split across
            # ACT and DVE) + output DMA ride the call that finishes a chunk.
            # At the tail the scores/po psum slots are idle, so chunks rotate
            # through all three pools and the drains overlap the matmuls.
            if proj_state[0] is None:
                if not pending_proj:
                    return False
                qb, ots, dc = pending_proj[0]
                if dc + 1 == KD:
                    pending_proj.pop(0)
                else:
                    pending_proj[0] = (qb, ots, dc + 1)
                if tail:
                    pool, tag = [(ps_x, "x"), (ps_s, "s"), (ps_o, "o")][tail_rot[0] % 3]
                    tail_rot[0] += 1
                else:
                    pool, tag = ps_x, "x"
                pp = pool.tile([P, QB], F32, tag=tag, name=f"pp{qb}_{dc}")
                proj_state[0] = (qb, ots, dc, pp, 0)
            qb, ots, dc, pp, h = proj_state[0]
            last_h = len(ots) - 1
            cols = slice(dc * P, (dc + 1) * P)
            if len(ots) == 2:
                lhsT = [wp01[:, cols], wp2[:, cols]][h]
                rhs = [ots[0][:, :], ots[1][:, :]][h]
            else:  # last qb, unstacked: no dependency on the ot01 DMA shift
                lhsT = [wp01[0:HD, cols], wp1[:, cols], wp2[:, cols]][h]
                rhs = [ots[0][0:HD, :], ots[1][:, :], ots[2][:, :]][h]
            nc.tensor.matmul(pp[:], lhsT, rhs, start=(h == 0), stop=(h == last_h))
            if h == last_h:
                # bf16 partials: halves the output DMA; the host f32-sums the
                # 4 per-batch partials so the rounding stays ~0.2% per term
                outsb = out_pool.tile([P, QB], BF16, tag="outsb")
                nc.scalar.copy(outsb[:, 0 : QB // 2], pp[:, 0 : QB // 2])
                nc.vector.tensor_copy(outsb[:, QB // 2 :], pp[:, QB // 2 :])
                nc.gpsimd.dma_start(
                    out=out_d[dc * P : (dc + 1) * P, qb * QB : (qb + 1) * QB],
                    in_=outsb[:].bitcast(mybir.dt.uint16),
                )
                proj_state[0] = None
            else:
                proj_state[0] = (qb, ots, dc, pp, h + 1)
            return True

        pending_norm = []  # (u, dsb2h, ocp) after normalize_a
        for p0 in range(0, len(units), 2):
            pair = units[p0 : p0 + 2]
            last_pair = p0 + 2 >= len(units)
            # one unit's Q/K at partition base 0, the other at base 64
            bases = []
            seen64 = False
            for qb, h in pair:
                if h == 1:
                    bases.append(True)
                    seen64 = True
                else:
                    bases.append(False)
            if not seen64:  # (h2, h0) pair: put h2 at base 64
                bases = [h == 2 for qb, h in pair]
            # free the previous pair's po slots first: emit their psum reads
            # now (ACT/DVE run them while the PE streams this pair's first
            # scores / the previous pair's last AV).
            normed = [(u, *normalize_a(u, po, i)) for i, (u, po) in enumerate(pending_norm)]
            pending_norm = []
            pos = [
                ps_o.tile([HD + 1, QB], F32, tag="o", name=f"po_{p0}_{i}")
                for i in range(len(pair))
            ]
            es_hist = []
            for g in range(NKC):
                # AV lags the scores by TWO rounds: exp(g) gets ~2 rounds of
                # slack before its es is consumed, and the new pair's first
                # AV lands after the old pair's po slots have recycled.
                if g >= 2:
                    emit_av(pair, pos, es_hist[g - 2], g - 2)
                es_hist.append(emit_scores(pair, bases, g))
                # previous pair's normalize tails ride rounds 2-3; one or two
                # projection matmuls bridge the other rounds (two when the
                # backlog is deep, so it never piles up into a cold tail).
                if g in (2, 3) and normed:
                    normalize_b(*normed.pop(0))
                else:
                    # on the last pair, reserve ~3 chunks of projection work
                    # so the PE stays busy (and HAM stays warm) through the
                    # tail's normalize chains
                    rem = sum(KD - dc for _, _, dc in pending_proj)
                    if not (last_pair and rem <= 2):
                        emit_proj_step()
                        rem = sum(KD - dc for _, _, dc in pending_proj)
                        if rem >= 4 or (proj_state[0] is not None and rem >= 2):
                            emit_proj_step()
            emit_av(pair, pos, es_hist[NKC - 2], NKC - 2)
            emit_av(pair, pos, es_hist[NKC - 1], NKC - 1)
            for u, po in zip(pair, pos):
                pending_norm.append((u, po))
        # tail: the last pair's norms run on ACT/DVE/gpsimd while the PE
        # chews the reserved projection chunks; the final qb then projects
        # as two dense waves (all h01 matmuls across 6 idle psum banks,
        # then all h2 matmuls + drains) so the PE never idles long enough
        # to get clock-throttled.
        normed = [(u, *normalize_a(u, po, i)) for i, (u, po) in enumerate(pending_norm)]
        while proj_state[0] is not None or pending_proj:
            emit_proj_step(tail=True)
        for item in normed:
            normalize_b(*item)
        assert len(pending_proj) == 1 and pending_proj[0][2] == 0
        qb, ots, _ = pending_proj.pop(0)
        slots = [(ps_x, "x"), (ps_s, "s"), (ps_s, "s"), (ps_s, "s"), (ps_o, "o"), (ps_o, "o")]
        pps = []
        for dc in range(KD):
            pool, tag = slots[dc]
            pp = pool.tile([P, QB], F32, tag=tag, name=f"ppw{qb}_{dc}")
            nc.tensor.matmul(
                pp[:], wp01[:, dc * P : (dc + 1) * P], ots[0][:, :],
                start=True, stop=False,
            )
            pps.append(pp)
        for dc in range(KD):
            nc.tensor.matmul(
                pps[dc][:], wp2[:, dc * P : (dc + 1) * P], ots[1][:, :],
                start=False, stop=True,
            )
            outsb = out_pool.tile([P, QB], BF16, tag="outsb")
            nc.scalar.copy(outsb[:, 0 : QB // 2], pps[dc][:, 0 : QB // 2])
            nc.vector.tensor_copy(outsb[:, QB // 2 :], pps[dc][:, QB // 2 :])
            nc.gpsimd.dma_start(
                out=out_d[dc * P : (dc + 1) * P, qb * QB : (qb + 1) * QB],
                in_=outsb[:].bitcast(mybir.dt.uint16),
            )

        attn_ctx.close()

    nc.compile()
    return nc


def get_nc():
    if "nc" not in _CACHE:
        _CACHE["nc"] = _build_nc()
    return _CACHE["nc"]


def _bf16_bits(a):
    u = np.ascontiguousarray(a, np.float32).view(np.uint32)
    return ((u + 0x7FFF + ((u >> 16) & 1)) >> 16).astype(np.uint16)


def shard_inputs(x, w_qkv, b_qkv, w_proj):
    x = np.asarray(x, np.float32)
    w_qkv = np.asarray(w_qkv, np.float32)
    b_qkv = np.asarray(b_qkv, np.float32)
    w_proj = np.asarray(w_proj, np.float32)
    Wq, Wk = w_qkv[:, :D], w_qkv[:, D : 2 * D]
    Wv = w_qkv[:, 2 * D :]
    bq, bk = b_qkv[:D], b_qkv[D : 2 * D]
    in_maps = []
    for c in range(NCORES):
        b = c // 4
        lo = HD * HPC * (c % 4)
        sl = slice(lo, lo + HPC * HD)
        bq_s = bq[sl] * SCALE
        bk_s = bk[sl]
        bias4 = np.zeros((P, 4), np.float32)
        bias4[:, 0] = bq_s[0:P]
        bias4[:HD, 1] = bq_s[P : P + HD]
        bias4[HD:, 1] = bq_s[P : P + HD]
        bias4[:, 2] = bk_s[0:P]
        bias4[:HD, 3] = bk_s[P : P + HD]
        bias4[HD:, 3] = bk_s[P : P + HD]
        in_maps.append(
            {
                "xT": _bf16_bits(x[b].T),
                "wqk": _bf16_bits(
                    np.concatenate([Wq[:, sl] * SCALE, Wk[:, sl]], axis=1)
                ),
                "bqk": bias4,
                "wv": _bf16_bits(Wv[:, sl]),
                "wp": _bf16_bits(w_proj[sl, :]),
            }
        )
    return in_maps


def assemble(outs, w_qkv, b_qkv, w_proj, b_proj):
    b_proj = np.asarray(b_proj, np.float32)
    bv = np.asarray(b_qkv, np.float32)[2 * D :]
    # device drops the V bias; bv contributes bv @ w_proj to every row
    bias = b_proj + bv @ np.asarray(w_proj, np.float32)
    def to_f32(o):
        o = np.asarray(o)
        if o.dtype == np.uint16:  # bf16 bit patterns
            return (o.astype(np.uint32) << 16).view(np.float32)
        return o.astype(np.float32)

    y = np.empty((B, S, D), np.float32)
    for b in range(B):
        acc = to_f32(outs[4 * b])
        for i in range(1, 4):
            acc = acc + to_f32(outs[4 * b + i])
        y[b] = acc.T + bias
    return y


def run(inputs, trace=False, **kw):
    nc = get_nc()
    in_maps = shard_inputs(
        inputs["x"], inputs["w_qkv"], inputs["b_qkv"], inputs["w_proj"]
    )
    res = run_bass_kernel_spmd(
        nc, in_maps, core_ids=list(range(NCORES)), trace=trace, **kw
    )
    outs = [r["out"] for r in res.results]
    return (
        assemble(outs, inputs["w_qkv"], inputs["b_qkv"], inputs["w_proj"], inputs["b_proj"]),
        res,
    )


def kernel(x, w_qkv, b_qkv, w_proj, b_proj):
    y, _ = run(
        {"x": x, "w_qkv": w_qkv, "b_qkv": b_qkv, "w_proj": w_proj, "b_proj": b_proj}
    )
    return y


# revision 55
# speedup vs baseline: 1.0027x; 1.0027x over previous
"""Multi-head attention (B=2, S=2048, D=768, H=12) on 8 TRN2 NeuronCores.

Sharding: data-parallel over batch x tensor-parallel over heads.
  core c -> batch c//4, heads 3*(c%4) .. 3*(c%4)+2
Each core computes its 3 heads end-to-end plus the partial output
projection (its 192 rows of w_proj), emitted TRANSPOSED as out^T [D, S].
Host sums the 4 partials per batch, transposes, and adds b_proj plus the
folded V-bias term (bv @ w_proj), so the device never touches bv.

Performance structure (per core):
  The 128x128 PE array is 16 independent 32x32 sub-arrays; matmuls whose
  (row_grp, col_grp) strips are disjoint run CONCURRENTLY.  tile_position
  auto-derives as (lhsT.base_partition, out.base_partition).
    - scores have K=64: a pair of units whose Q/K live at SBUF bases (0, 64)
      co-runs 2x.  h0 lives at base 0, h1 at base 64, and h2 is produced at
      BOTH bases so score pairs always split rows.
    - AV has K=128: split into K=64 halves (v1/es partition halves); the lo
      half of one unit co-runs with the hi half of the other.
  Steady state runs one 512-column key-chunk per round: scores pair (216ns)
  + 4 AV half-MMs (432ns) + interleaved proj/broadcast work.  PSUM budget
  (16KB/partition = 8 banks): scores tag "s" 3x1 bank (so scores(g+1) never
  waits on exp(g-1) -- the psum-recycle recurrence that serialized the old
  kernel), po tag "o" 2x2 banks, aux tag "x" 1x1 bank (proj psum + the
  1->64 reciprocal broadcast).
  exp alternates per round between ACT (table exp) and DVE (Schraudolph:
  scores*128/ln2 + bias -> int16 == bf16 bits of ~exp; ~1.8% rms on half
  the groups), keeping each engine under the PE round time.
  Normalize is split: po psum reads (DVE) are emitted at the pair boundary
  so the po slots recycle immediately; the reciprocal row is broadcast
  1->64 rows with a warm PE matmul and the final multiply is a single
  scalar_tensor_tensor.  The PE never idles long enough for the HAM clock
  gate to re-throttle it to 1.2GHz (which cost the old kernel ~96us of
  half-clock time).
"""

from contextlib import ExitStack

import numpy as np

import concourse.bass as bass
import concourse.mybir as mybir
import concourse.tile as tile
from concourse import bacc
from concourse.bass_utils import run_bass_kernel_spmd

B, S, D = 2, 2048, 768
H, HD = 12, 64
SCALE = HD**-0.5
NCORES = 8
HPC = 3  # heads per core
P = 128
KD = D // P  # 6 chunks of contraction dim for projections
QB = 512  # query block (free dim of score matmuls)
NQB = S // QB  # 4
NKC = S // P  # 16 key chunks == rounds per pair
F32 = mybir.dt.float32
BF16 = mybir.dt.bfloat16
I16 = mybir.dt.int16
EXP = mybir.ActivationFunctionType.Exp
MULT = mybir.AluOpType.mult
ADD = mybir.AluOpType.add

# Schraudolph exp-in-bf16-bits: i16 = round(s*A16 + B16) are the bf16 bits
# of ~exp(s).  B16 = 127*128 - 7.5 centers the piecewise-linear error.
A16 = 128.0 / np.log(2.0)
B16 = 16256.0 - 7.5

_CACHE = {}


def _build_nc():
    nc = bacc.Bacc("TRN2", target_bir_lowering=False, debug=False)
    # x / weight tensors arrive as uint16 = bf16 bit patterns (host-side
    # round-to-nearest conversion) — half the HBM stream of f32.
    U16 = mybir.dt.uint16
    xT_d = nc.dram_tensor("xT", [D, S], U16, kind="ExternalInput").ap()
    wqk_d = nc.dram_tensor("wqk", [D, 2 * HPC * HD], U16, kind="ExternalInput").ap()
    bqk_d = nc.dram_tensor("bqk", [P, 4], F32, kind="ExternalInput").ap()
    wv_d = nc.dram_tensor("wv", [D, HPC * HD], U16, kind="ExternalInput").ap()
    wp_d = nc.dram_tensor("wp", [HPC * HD, D], U16, kind="ExternalInput").ap()
    # uint16 = bf16 bit patterns (the axon PJRT bridge can't move bf16)
    out_d = nc.dram_tensor("out", [D, S], mybir.dt.uint16, kind="ExternalOutput").ap()

    with tile.TileContext(nc) as tc, ExitStack() as ctx:
        const = ctx.enter_context(tc.tile_pool(name="const", bufs=1))
        stage = ctx.enter_context(tc.tile_pool(name="stage", bufs=2))
        es_pool = ctx.enter_context(tc.tile_pool(name="es", bufs=8))
        ot_pool = ctx.enter_context(tc.tile_pool(name="ot", bufs=3))
        rc_pool = ctx.enter_context(tc.tile_pool(name="rc", bufs=3))
        out_pool = ctx.enter_context(tc.tile_pool(name="outsb", bufs=4))

        # ---- load inputs ----
        # Direct DMA into the const tiles (uint16 bitcast = bf16 bits), in
        # consumption order: xt chunk 0 first so the QKV matmuls start ~5us
        # in and pipeline behind the rest of the x stream.
        bqk_sb = const.tile([P, 4], F32, tag="bqk")
        nc.gpsimd.dma_start(out=bqk_sb[:], in_=bqk_d[:, :])
        # warm the ACT exp table while DMAs stream
        warm = const.tile([1, 2], BF16, tag="warm")
        nc.vector.memset(warm[:], 0.0)
        nc.scalar.activation(warm[:], warm[:], EXP)
        ones_bf = const.tile([HD + 1, HD], BF16, tag="ones")
        nc.vector.memset(ones_bf[:], 1.0)

        # Matmul-input tiles are staged DMA -> stage tile -> DVE/ACT copy, so
        # every matmul's input-ready dependency is a cheap engine semaphore
        # (waiting on raw DMA semaphores measured ~+60ns on EVERY matmul).
        xt = [const.tile([P, S], BF16, tag=f"xt{i}", name=f"xt{i}") for i in range(KD)]
        wqk_sb = [
            const.tile([P, 2 * HPC * HD], BF16, tag=f"wqk{i}", name=f"wqk{i}")
            for i in range(KD)
        ]
        wv_sb = [
            const.tile([P, HPC * HD], BF16, tag=f"wv{i}", name=f"wv{i}")
            for i in range(KD)
        ]
        U16 = mybir.dt.uint16

        def load(dma_eng, dst, src_ap, kind, eng):
            st_t = stage.tile(list(dst.shape), BF16, tag=f"stg_{kind}")
            dma_eng.dma_start(out=st_t[:].bitcast(U16), in_=src_ap)
            if eng == "act":
                nc.scalar.copy(dst[:], st_t[:])
            else:
                nc.vector.tensor_copy(dst[:], st_t[:])

        # x loads are split per (chunk, 512-col block), first QKV bank's six
        # pieces first: bank nb computes while bank nb+1's pieces stream in.
        def load_xt_piece(i, nb):
            cols = slice(nb * QB, (nb + 1) * QB)
            st_t = stage.tile([P, QB], BF16, tag="stg_xt", bufs=8)
            (nc.sync if (i + nb) % 2 == 0 else nc.gpsimd).dma_start(
                out=st_t[:].bitcast(U16), in_=xT_d[i * P : (i + 1) * P, cols]
            )
            if (i + nb) % 2:
                nc.scalar.copy(xt[i][:, cols], st_t[:])
            else:
                nc.vector.tensor_copy(xt[i][:, cols], st_t[:])

        for i in range(KD):
            load_xt_piece(i, 0)
            if i < 3:
                load(nc.gpsimd, wqk_sb[i], wqk_d[i * P : (i + 1) * P, :], "wqk", "act")
        for i in range(3, KD):
            load(nc.gpsimd, wqk_sb[i], wqk_d[i * P : (i + 1) * P, :], "wqk", "act")
        for nb in range(1, NQB):
            for i in range(KD):
                load_xt_piece(i, nb)
        for i in range(KD):
            load(nc.sync, wv_sb[i], wv_d[i * P : (i + 1) * P, :], "wv", "dve")
        # output projection weights: h0+h1 stacked on all 128 partitions so
        # each proj chunk is one K=128 matmul plus a K=64 for h2
        wp01 = const.tile([P, D], BF16, tag="wp01")
        load(nc.gpsimd, wp01, wp_d[0 : 2 * HD, :], "wp01", "act")
        wp2 = const.tile([HD, D], BF16, tag="wp2")
        load(nc.gpsimd, wp2, wp_d[2 * HD : 3 * HD, :], "wp2", "act")
        # base-0 copy of h1's rows: the LAST qb projects unstacked (3 K=64
        # matmuls) so its chunks never wait on the ot01 DMA partition-shift
        wp1 = const.tile([HD, D], BF16, tag="wp1")
        load(nc.gpsimd, wp1, wp_d[HD : 2 * HD, :], "wp1", "act")

        # v1 tiles ([v_h | ones] per head) memset early, during the loads
        v1 = []
        for st in range(NKC):
            t = const.tile([P, HPC * (HD + 1)], BF16, tag=f"v1_{st}")
            nc.vector.memset(t[:], 1.0)
            v1.append(t)

        # ---- Q^T / K^T projection, k-outer in 2 passes of 8 psum banks ----
        # h2's Q/K are produced at BOTH partition bases so score pairs can
        # always put one lhsT at rows 0-63 and the other at rows 64-127.
        tq0 = const.tile([P, S], BF16, tag="tq0")  # [q_h0; q_h1]
        tk0 = const.tile([P, S], BF16, tag="tk0")  # [k_h0; k_h1]
        tq1 = const.tile([HD, S], BF16, tag="tq1")  # q_h2 @ base 0
        tk1 = const.tile([HD, S], BF16, tag="tk1")  # k_h2 @ base 0
        tq1h = const.tile([P, S], BF16, tag="tq1h")  # rows 64:128 = q_h2
        tk1h = const.tile([P, S], BF16, tag="tk1h")  # rows 64:128 = k_h2
        CQ2, CK2 = P, 2 * P + HD  # wqk col starts of q2 / k2
        # pass -> (full-m spec, colpair spec): colpair writes lo/hi halves of
        # one psum bank concurrently (auto tile_position (0,0) / (0,64)).
        passes = [
            ((tq0, 0, 0), (tq1, CQ2, 1, tk1h, CK2, 3)),
            ((tk0, P + HD, 2), (tk1, CK2, 3, tq1h, CQ2, 1)),
        ]
        qk_ctx = ExitStack()
        ps_qk = qk_ctx.enter_context(tc.tile_pool(name="ps_qk", bufs=8, space="PSUM"))
        for (dst0, c00, b0), (dlo, clo, blo, dhi, chi, bhi) in passes:
            # bank-outer / k-inner: each nb's psum banks finish early so the
            # bias-add drains overlap the next bank's matmuls instead of
            # piling up (PE-idle) at the end of the pass
            for nb in range(NQB):
                psf = ps_qk.tile([P, QB], F32, tag="qk", name=f"psf{b0}_{nb}")
                psc = ps_qk.tile([P, QB], F32, tag="qk", name=f"psc{b0}_{nb}")
                for k in range(KD):
                    x_ap = xt[k][:, nb * QB : (nb + 1) * QB]
                    st = dict(start=(k == 0), stop=(k == KD - 1))
                    nc.tensor.matmul(psf[:], wqk_sb[k][:, c00 : c00 + P], x_ap, **st)
                    nc.tensor.matmul(psc[0:HD], wqk_sb[k][:, clo : clo + HD], x_ap, **st)
                    nc.tensor.matmul(psc[HD:P], wqk_sb[k][:, chi : chi + HD], x_ap, **st)
                cols = slice(nb * QB, (nb + 1) * QB)
                # bias-adds drain the psum banks, balanced across ACT and DVE
                if nb % 2:
                    nc.vector.tensor_scalar_add(dst0[:, cols], psf[:], bqk_sb[:, b0 : b0 + 1])
                    nc.vector.tensor_scalar_add(dlo[:, cols], psc[0:HD], bqk_sb[0:HD, blo : blo + 1])
                    nc.scalar.activation(dhi[HD:P, cols], psc[HD:P],
                                         mybir.ActivationFunctionType.Identity,
                                         bias=bqk_sb[HD:P, bhi : bhi + 1])
                else:
                    nc.scalar.activation(dst0[:, cols], psf[:],
                                         mybir.ActivationFunctionType.Identity,
                                         bias=bqk_sb[:, b0 : b0 + 1])
                    nc.scalar.activation(dlo[:, cols], psc[0:HD],
                                         mybir.ActivationFunctionType.Identity,
                                         bias=bqk_sb[0:HD, blo : blo + 1])
                    nc.vector.tensor_scalar_add(dhi[HD:P, cols], psc[HD:P], bqk_sb[HD:P, bhi : bhi + 1])
        qk_ctx.close()

        # ---- V projection, k-outer in 2 passes of 8 key chunks ----
        v_ctx = ExitStack()
        ps_v = v_ctx.enter_context(tc.tile_pool(name="ps_v", bufs=8, space="PSUM"))
        for pas in (range(0, 8), range(8, NKC)):
            psl = [ps_v.tile([P, HPC * HD], F32, tag="v", name=f"pv{st}") for st in pas]
            # chunk-outer so each psum bank finishes (and its drain + the
            # next tenant's matmuls start) without waiting for the whole pass
            for st, pv in zip(pas, psl):
                for k in range(KD):
                    nc.tensor.matmul(
                        pv[:],
                        xt[k][:, st * P : (st + 1) * P],
                        wv_sb[k][:],
                        start=(k == 0),
                        stop=(k == KD - 1),
                    )
                # drain psum -> v1 head blocks, alternating DVE/ACT per chunk
                for h in range(HPC):
                    dst = v1[st][:, h * (HD + 1) : h * (HD + 1) + HD]
                    src = pv[:, h * HD : (h + 1) * HD]
                    if st % 2:
                        nc.scalar.copy(dst, src)
                    else:
                        nc.vector.tensor_copy(dst, src)
        v_ctx.close()

        # ---- attention steady state ----
        attn_ctx = ExitStack()
        ps_s = attn_ctx.enter_context(tc.tile_pool(name="ps_s", bufs=3, space="PSUM"))
        ps_o = attn_ctx.enter_context(tc.tile_pool(name="ps_o", bufs=4, space="PSUM"))
        ps_x = attn_ctx.enter_context(tc.tile_pool(name="ps_x", bufs=1, space="PSUM"))

        def qk_loc(h, base64):
            # -> (q tile, k tile, partition offset)
            if h == 0:
                return tq0, tk0, 0
            if h == 1:
                return tq0, tk0, HD
            return (tq1h, tk1h, HD) if base64 else (tq1, tk1, 0)

        units = [(qb, h) for qb in range(NQB) for h in range(HPC)]

        def emit_scores(pair, bases, g):
            # one key chunk per round: 2 score MMs co-run (rows split via the
            # 0/64 lhsT bases), then one exp per engine (ACT / DVE alternate
            # per round so neither engine saturates).
            ess = []
            for i, ((qb, h), b64) in enumerate(zip(pair, bases)):
                qt, kt, o = qk_loc(h, b64)
                ps = ps_s.tile([P, QB], F32, tag="s", name=f"ps{qb}_{h}_{g}")
                nc.tensor.matmul(
                    ps[:],
                    kt[o : o + HD, g * P : (g + 1) * P],
                    qt[o : o + HD, qb * QB : (qb + 1) * QB],
                    start=True,
                    stop=True,
                )
                es = es_pool.tile([P, QB], BF16, tag="es", name=f"es{qb}_{h}_{g}")
                if (g + i) % 2:
                    nc.vector.tensor_scalar(
                        out=es[:].bitcast(I16), in0=ps[:], scalar1=A16, scalar2=B16,
                        op0=MULT, op1=ADD,
                    )
                else:
                    nc.scalar.activation(es[:], ps[:], EXP)
                ess.append(es)
            return ess

        def emit_av(pair, pos, ess, kc):
            # one K=128 matmul per unit into a single-region po [65, 512]:
            # row 64 (the v1 ones column) accumulates the softmax denominator
            # in place, so normalize needs no halves-add.
            for (qb, h), po, es in zip(pair, pos, ess):
                nc.tensor.matmul(
                    po[:],
                    v1[kc][:, h * (HD + 1) : (h + 1) * (HD + 1)],
                    es[:],
                    start=(kc == 0),
                    stop=(kc == NKC - 1),
                )

        def normalize_a(u, po, i):
            # All the po (psum) readers, emitted at the pair boundary so the
            # po slot recycles before the next pair's AV needs it.  One
            # [1,512] bf16-converting denom-row copy + one [64,512] copy,
            # split across ACT/DVE (alternating by unit).
            dsb2h = rc_pool.tile([HD + 1, QB], BF16, tag="rc2")
            ocp = rc_pool.tile([HD, QB], F32, tag="ocp")
            if i % 2:
                nc.scalar.copy(dsb2h[HD : HD + 1, :], po[HD : HD + 1, :])
                nc.vector.tensor_copy(ocp[:], po[0:HD, :])
            else:
                nc.vector.tensor_copy(dsb2h[HD : HD + 1, :], po[HD : HD + 1, :])
                nc.scalar.copy(ocp[:], po[0:HD, :])
            return dsb2h, ocp

        ot01_map = {}  # qb -> stacked [h0; h1] ot tile

        def normalize_b(u, dsb2h, ocp):
            # broadcast the denom from row 64 to rows 0-63 on the PE (bf16
            # matmul — stays in the matmul stream, keeps HAM warm);
            # reciprocal must run at base partition 0 (custom DVE ops are
            # silent no-ops elsewhere).  h0/h1 outputs land stacked on
            # partitions 0-63 / 64-127 (h1 via an SBUF->SBUF DMA partition
            # shift, off the critical path) so each projection chunk is one
            # K=128 matmul + one K=64 instead of three K=64.
            qb, h = u
            prb = ps_x.tile([HD, QB], F32, tag="x", name=f"prb{qb}_{h}")
            nc.tensor.matmul(
                prb[:], ones_bf[HD : HD + 1, :], dsb2h[HD : HD + 1, :],
                start=True, stop=True,
            )
            rb = rc_pool.tile([HD, QB], F32, tag="rb")
            nc.vector.reciprocal_approx_fast(rb[:], prb[:])
            if h == 0:
                ot01 = ot_pool.tile([P, QB], BF16, tag="ot01", name=f"ot01_{qb}")
                ot01_map[qb] = [ot01]
                nc.gpsimd.tensor_tensor(out=ot01[0:HD, :], in0=ocp[:], in1=rb[:], op=MULT)
            elif h == 1:
                tmp = ot_pool.tile([HD, QB], BF16, tag="otmp", name=f"otmp_{qb}")
                nc.gpsimd.tensor_tensor(out=tmp[:], in0=ocp[:], in1=rb[:], op=MULT)
                nc.sync.dma_start(out=ot01_map[qb][0][HD:P, :], in_=tmp[:])
            else:
                ot2 = ot_pool.tile([HD, QB], BF16, tag="ot2", name=f"ot2_{qb}")
                nc.gpsimd.tensor_tensor(out=ot2[:], in0=ocp[:], in1=rb[:], op=MULT)
                pending_proj.append((qb, (*ot01_map.pop(qb), ot2), 0))

        pending_proj = []  # (qb, [ot_h0, ot_h1, ot_h2], next_dchunk)
        proj_state = [None]  # (qb, ots, dc, pp, next_h) mid-chunk
        tail_rot = [0]

        def emit_proj_step(tail=False):
            # one projection matmul per call; the psum drain (split across
            # ACT and DVE) + output DMA ride the call that finishes a chunk.
            # At the tail the scores/po psum slots are idle, so chunks rotate
            # through all three pools and the drains overlap the matmuls.
            if proj_state[0] is None:
                if not pending_proj:
                    return False
                qb, ots, dc = pending_proj[0]
                if dc + 1 == KD:
                    pending_proj.pop(0)
                else:
                    pending_proj[0] = (qb, ots, dc + 1)
                if tail:
                    pool, tag = [(ps_x, "x"), (ps_s, "s"), (ps_o, "o")][tail_rot[0] % 3]
                    tail_rot[0] += 1
                else:
                    pool, tag = ps_x, "x"
                pp = pool.tile([P, QB], F32, tag=tag, name=f"pp{qb}_{dc}")
                proj_state[0] = (qb, ots, dc, pp, 0)
            qb, ots, dc, pp, h = proj_state[0]
            last_h = len(ots) - 1
            cols = slice(dc * P, (dc + 1) * P)
            if len(ots) == 2:
                lhsT = [wp01[:, cols], wp2[:, cols]][h]
                rhs = [ots[0][:, :], ots[1][:, :]][h]
            else:  # last qb, unstacked: no dependency on the ot01 DMA shift
                lhsT = [wp01[0:HD, cols], wp1[:, cols], wp2[:, cols]][h]
                rhs = [ots[0][0:HD, :], ots[1][:, :], ots[2][:, :]][h]
            nc.tensor.matmul(pp[:], lhsT, rhs, start=(h == 0), stop=(h == last_h))
            if h == last_h:
                # bf16 partials: halves the output DMA; the host f32-sums the
                # 4 per-batch partials so the rounding stays ~0.2% per term
                outsb = out_pool.tile([P, QB], BF16, tag="outsb")
                nc.scalar.copy(outsb[:, 0 : QB // 2], pp[:, 0 : QB // 2])
                nc.vector.tensor_copy(outsb[:, QB // 2 :], pp[:, QB // 2 :])
                nc.gpsimd.dma_start(
                    out=out_d[dc * P : (dc + 1) * P, qb * QB : (qb + 1) * QB],
                    in_=outsb[:].bitcast(mybir.dt.uint16),
                )
                proj_state[0] = None
            else:
                proj_state[0] = (qb, ots, dc, pp, h + 1)
            return True

        pending_norm = []  # (u, dsb2h, ocp) after normalize_a
        for p0 in range(0, len(units), 2):
            pair = units[p0 : p0 + 2]
            last_pair = p0 + 2 >= len(units)
            # one unit's Q/K at partition base 0, the other at base 64
            bases = []
            seen64 = False
            for qb, h in pair:
                if h == 1:
                    bases.append(True)
                    seen64 = True
                else:
                    bases.append(False)
            if not seen64:  # (h2, h0) pair: put h2 at base 64
                bases = [h == 2 for qb, h in pair]
            # free the previous pair's po slots first: emit their psum reads
            # now (ACT/DVE run them while the PE streams this pair's first
            # scores / the previous pair's last AV).
            normed = [(u, *normalize_a(u, po, i)) for i, (u, po) in enumerate(pending_norm)]
            pending_norm = []
            pos = [
                ps_o.tile([HD + 1, QB], F32, tag="o", name=f"po_{p0}_{i}")
                for i in range(len(pair))
            ]
            es_hist = []
            for g in range(NKC):
                # AV lags the scores by TWO rounds: exp(g) gets ~2 rounds of
                # slack before its es is consumed, and the new pair's first
                # AV lands after the old pair's po slots have recycled.
                if g >= 2:
                    emit_av(pair, pos, es_hist[g - 2], g - 2)
                es_hist.append(emit_scores(pair, bases, g))
                # previous pair's normalize tails ride rounds 2-3; one or two
                # projection matmuls bridge the other rounds (two when the
                # backlog is deep, so it never piles up into a cold tail).
                if g in (2, 3) and normed:
                    normalize_b(*normed.pop(0))
                else:
                    # on the last pair, reserve ~3 chunks of projection work
                    # so the PE stays busy (and HAM stays warm) through the
                    # tail's normalize chains
                    rem = sum(KD - dc for _, _, dc in pending_proj)
                    if not (last_pair and rem <= 2):
                        emit_proj_step()
                        rem = sum(KD - dc for _, _, dc in pending_proj)
                        if rem >= 4 or (proj_state[0] is not None and rem >= 2):
                            emit_proj_step()
            emit_av(pair, pos, es_hist[NKC - 2], NKC - 2)
            emit_av(pair, pos, es_hist[NKC - 1], NKC - 1)
            for u, po in zip(pair, pos):
                pending_norm.append((u, po))
        # tail: the last pair's norms run on ACT/DVE/gpsimd while the PE
        # chews the reserved projection chunks; the final qb then projects
        # as two dense waves (all h01 matmuls across 6 idle psum banks,
        # then all h2 matmuls + drains) so the PE never idles long enough
        # to get clock-throttled.
        normed = [(u, *normalize_a(u, po, i)) for i, (u, po) in enumerate(pending_norm)]
        while proj_state[0] is not None or pending_proj:
            emit_proj_step(tail=True)
        for item in normed:
            normalize_b(*item)
        assert len(pending_proj) == 1 and pending_proj[0][2] == 0
        qb, ots, _ = pending_proj.pop(0)
        slots = [(ps_x, "x"), (ps_s, "s"), (ps_s, "s"), (ps_s, "s"), (ps_o, "o"), (ps_o, "o")]
        pps = []
        for dc in range(KD):
            pool, tag = slots[dc]
            pp = pool.tile([P, QB], F32, tag=tag, name=f"ppw{qb}_{dc}")
            nc.tensor.matmul(
                pp[:], wp01[:, dc * P : (dc + 1) * P], ots[0][:, :],
                start=True, stop=False,
            )
            pps.append(pp)
        for dc in range(KD):
            nc.tensor.matmul(
                pps[dc][:], wp2[:, dc * P : (dc + 1) * P], ots[1][:, :],
                start=False, stop=True,
            )
            outsb = out_pool.tile([P, QB], BF16, tag="outsb")
            nc.scalar.copy(outsb[:, 0 : QB // 2], pps[dc][:, 0 : QB // 2])
            nc.vector.tensor_copy(outsb[:, QB // 2 :], pps[dc][:, QB // 2 :])
            nc.gpsimd.dma_start(
                out=out_d[dc * P : (dc + 1) * P, qb * QB : (qb + 1) * QB],
                in_=outsb[:].bitcast(mybir.dt.uint16),
            )

        attn_ctx.close()

    nc.compile()
    return nc


def get_nc():
    if "nc" not in _CACHE:
        _CACHE["nc"] = _build_nc()
    return _CACHE["nc"]


def _bf16_bits(a):
    u = np.ascontiguousarray(a, np.float32).view(np.uint32)
    return ((u + 0x7FFF + ((u >> 16) & 1)) >> 16).astype(np.uint16)


def shard_inputs(x, w_qkv, b_qkv, w_proj):
    x = np.asarray(x, np.float32)
    w_qkv = np.asarray(w_qkv, np.float32)
    b_qkv = np.asarray(b_qkv, np.float32)
    w_proj = np.asarray(w_proj, np.float32)
    Wq, Wk = w_qkv[:, :D], w_qkv[:, D : 2 * D]
    Wv = w_qkv[:, 2 * D :]
    bq, bk = b_qkv[:D], b_qkv[D : 2 * D]
    in_maps = []
    for c in range(NCORES):
        b = c // 4
        lo = HD * HPC * (c % 4)
        sl = slice(lo, lo + HPC * HD)
        bq_s = bq[sl] * SCALE
        bk_s = bk[sl]
        bias4 = np.zeros((P, 4), np.float32)
        bias4[:, 0] = bq_s[0:P]
        bias4[:HD, 1] = bq_s[P : P + HD]
        bias4[HD:, 1] = bq_s[P : P + HD]
        bias4[:, 2] = bk_s[0:P]
        bias4[:HD, 3] = bk_s[P : P + HD]
        bias4[HD:, 3] = bk_s[P : P + HD]
        in_maps.append(
            {
                "xT": _bf16_bits(x[b].T),
                "wqk": _bf16_bits(
                    np.concatenate([Wq[:, sl] * SCALE, Wk[:, sl]], axis=1)
                ),
                "bqk": bias4,
                "wv": _bf16_bits(Wv[:, sl]),
                "wp": _bf16_bits(w_proj[sl, :]),
            }
        )
    return in_maps


def assemble(outs, w_qkv, b_qkv, w_proj, b_proj):
    b_proj = np.asarray(b_proj, np.float32)
    bv = np.asarray(b_qkv, np.float32)[2 * D :]
    # device drops the V bias; bv contributes bv @ w_proj to every row
    bias = b_proj + bv @ np.asarray(w_proj, np.float32)
    def to_f32(o):
        o = np.asarray(o)
        if o.dtype == np.uint16:  # bf16 bit patterns
            return (o.astype(np.uint32) << 16).view(np.float32)
        return o.astype(np.float32)

    y = np.empty((B, S, D), np.float32)
    for b in range(B):
        acc = to_f32(outs[4 * b])
        for i in range(1, 4):
            acc = acc + to_f32(outs[4 * b + i])
        y[b] = acc.T + bias
    return y


def run(inputs, trace=False, **kw):
    nc = get_nc()
    in_maps = shard_inputs(
        inputs["x"], inputs["w_qkv"], inputs["b_qkv"], inputs["w_proj"]
    )
    res = run_bass_kernel_spmd(
        nc, in_maps, core_ids=list(range(NCORES)), trace=trace, **kw
    )
    outs = [r["out"] for r in res.results]
    return (
        assemble(outs, inputs["w_qkv"], inputs["b_qkv"], inputs["w_proj"], inputs["b_proj"]),
        res,
    )


def kernel(x, w_qkv, b_qkv, w_proj, b_proj):
    y, _ = run(
        {"x": x, "w_qkv": w_qkv, "b_qkv": b_qkv, "w_proj": w_proj, "b_proj": b_proj}
    )
    return y


# revision 57
# speedup vs baseline: 1.0098x; 1.0071x over previous
"""Multi-head attention (B=2, S=2048, D=768, H=12) on 8 TRN2 NeuronCores.

Sharding: data-parallel over batch x tensor-parallel over heads.
  core c -> batch c//4, heads 3*(c%4) .. 3*(c%4)+2
Each core computes its 3 heads end-to-end plus the partial output
projection (its 192 rows of w_proj), emitted TRANSPOSED as out^T [D, S].
Host sums the 4 partials per batch, transposes, and adds b_proj plus the
folded V-bias term (bv @ w_proj), so the device never touches bv.

Performance structure (per core):
  The 128x128 PE array is 16 independent 32x32 sub-arrays; matmuls whose
  (row_grp, col_grp) strips are disjoint run CONCURRENTLY.  tile_position
  auto-derives as (lhsT.base_partition, out.base_partition).  Scores have
  K=64: a pair of units whose Q/K live at SBUF bases (0, 64) co-runs 2x
  (h0 at base 0, h1 at base 64, h2 produced at BOTH bases so pairs always
  split rows).

  Steady state = 96 rounds (6 unit-pairs x 16 key chunks), per round:
  scores pair (1 co-run slot) + 2 single K=128 AV matmuls into po [65,512]
  (the v1 ones-column accumulates the softmax denominator in row 64 in
  place) + at most one interleaved projection/broadcast matmul.  AV lags
  the scores by 2 rounds so exp latency is never on the PE critical path.
  PSUM (16KB/partition = 8 banks): scores "s" 3x1 (scores(g+1) never waits
  on exp(g-1) -- the psum-recycle recurrence that serialized the original
  kernel), po "o" 4x1, aux "x" 1x1 (proj psum + reciprocal broadcast).
  exp alternates per round between ACT (table exp) and DVE (Schraudolph:
  scores*128/ln2 + bias -> int16 == bf16 bits of ~exp, on half the tiles).
  Normalize: po psum reads are emitted at the pair boundary (ACT/DVE
  split; the [1,512] denom row costs ~600ns because single-partition ops
  run at ~1 elem/cycle TOTAL) so po slots recycle fast; the denom is
  broadcast 1->64 rows with a bf16 PE matmul (fp32 matmuls are ~4x
  slower), reciprocal'd at base partition 0 (custom DVE ops silently
  no-op at other bases), multiplied on gpsimd.  ot h0/h1 land stacked on
  partitions 0-63/64-127 (h1 via an SBUF->SBUF DMA partition shift) so
  each projection chunk is one K=128 + one K=64 matmul; the outsb drains
  are column-split across ACT and DVE; output partials leave as bf16
  (uint16 bits).  The final qb projects as two dense waves across six
  idle psum banks at the tail.  Net effect: the HAM clock gate stays at
  8/8 (2.4GHz) from ~20us to the tail -- half-clock throttling cost the
  original kernel ~96us.
"""

from contextlib import ExitStack

import numpy as np

import concourse.bass as bass
import concourse.mybir as mybir
import concourse.tile as tile
from concourse import bacc
from concourse.bass_utils import run_bass_kernel_spmd

B, S, D = 2, 2048, 768
H, HD = 12, 64
SCALE = HD**-0.5
NCORES = 8
HPC = 3  # heads per core
P = 128
KD = D // P  # 6 chunks of contraction dim for projections
QB = 512  # query block (free dim of score matmuls)
NQB = S // QB  # 4
NKC = S // P  # 16 key chunks == rounds per pair
F32 = mybir.dt.float32
BF16 = mybir.dt.bfloat16
I16 = mybir.dt.int16
EXP = mybir.ActivationFunctionType.Exp
MULT = mybir.AluOpType.mult
ADD = mybir.AluOpType.add

# Schraudolph exp-in-bf16-bits: i16 = round(s*A16 + B16) are the bf16 bits
# of ~exp(s).  B16 = 127*128 - 7.5 centers the piecewise-linear error.
A16 = 128.0 / np.log(2.0)
B16 = 16256.0 - 7.5

_CACHE = {}


def _build_nc():
    nc = bacc.Bacc("TRN2", target_bir_lowering=False, debug=False)
    # x / weight tensors arrive as uint16 = bf16 bit patterns (host-side
    # round-to-nearest conversion) — half the HBM stream of f32.
    U16 = mybir.dt.uint16
    xT_d = nc.dram_tensor("xT", [D, S], U16, kind="ExternalInput").ap()
    wqk_d = nc.dram_tensor("wqk", [D, 2 * HPC * HD], U16, kind="ExternalInput").ap()
    bqk_d = nc.dram_tensor("bqk", [P, 4], F32, kind="ExternalInput").ap()
    wv_d = nc.dram_tensor("wv", [D, HPC * HD], U16, kind="ExternalInput").ap()
    wp_d = nc.dram_tensor("wp", [HPC * HD, D], U16, kind="ExternalInput").ap()
    # uint16 = bf16 bit patterns (the axon PJRT bridge can't move bf16)
    out_d = nc.dram_tensor("out", [D, S], mybir.dt.uint16, kind="ExternalOutput").ap()

    with tile.TileContext(nc) as tc, ExitStack() as ctx:
        const = ctx.enter_context(tc.tile_pool(name="const", bufs=1))
        stage = ctx.enter_context(tc.tile_pool(name="stage", bufs=2))
        es_pool = ctx.enter_context(tc.tile_pool(name="es", bufs=8))
        ot_pool = ctx.enter_context(tc.tile_pool(name="ot", bufs=3))
        rc_pool = ctx.enter_context(tc.tile_pool(name="rc", bufs=3))
        out_pool = ctx.enter_context(tc.tile_pool(name="outsb", bufs=4))

        # ---- load inputs ----
        # Direct DMA into the const tiles (uint16 bitcast = bf16 bits), in
        # consumption order: xt chunk 0 first so the QKV matmuls start ~5us
        # in and pipeline behind the rest of the x stream.
        bqk_sb = const.tile([P, 4], F32, tag="bqk")
        nc.gpsimd.dma_start(out=bqk_sb[:], in_=bqk_d[:, :])
        # warm the ACT exp table while DMAs stream
        warm = const.tile([1, 2], BF16, tag="warm")
        nc.vector.memset(warm[:], 0.0)
        nc.scalar.activation(warm[:], warm[:], EXP)
        ones_bf = const.tile([HD + 1, HD], BF16, tag="ones")
        nc.vector.memset(ones_bf[:], 1.0)

        # Matmul-input tiles are staged DMA -> stage tile -> DVE/ACT copy, so
        # every matmul's input-ready dependency is a cheap engine semaphore
        # (waiting on raw DMA semaphores measured ~+60ns on EVERY matmul).
        xt = [const.tile([P, S], BF16, tag=f"xt{i}", name=f"xt{i}") for i in range(KD)]
        wqk_sb = [
            const.tile([P, 2 * HPC * HD], BF16, tag=f"wqk{i}", name=f"wqk{i}")
            for i in range(KD)
        ]
        wv_sb = [
            const.tile([P, HPC * HD], BF16, tag=f"wv{i}", name=f"wv{i}")
            for i in range(KD)
        ]
        U16 = mybir.dt.uint16

        def load(dma_eng, dst, src_ap, kind, eng):
            st_t = stage.tile(list(dst.shape), BF16, tag=f"stg_{kind}")
            dma_eng.dma_start(out=st_t[:].bitcast(U16), in_=src_ap)
            if eng == "act":
                nc.scalar.copy(dst[:], st_t[:])
            else:
                nc.vector.tensor_copy(dst[:], st_t[:])

        # x loads are split per (chunk, 512-col block), first QKV bank's six
        # pieces first: bank nb computes while bank nb+1's pieces stream in.
        def load_xt_piece(i, nb):
            cols = slice(nb * QB, (nb + 1) * QB)
            st_t = stage.tile([P, QB], BF16, tag="stg_xt", bufs=8)
            (nc.sync if (i + nb) % 2 == 0 else nc.gpsimd).dma_start(
                out=st_t[:].bitcast(U16), in_=xT_d[i * P : (i + 1) * P, cols]
            )
            if (i + nb) % 2:
                nc.scalar.copy(xt[i][:, cols], st_t[:])
            else:
                nc.vector.tensor_copy(xt[i][:, cols], st_t[:])

        for i in range(KD):
            load_xt_piece(i, 0)
            if i < 3:
                load(nc.gpsimd, wqk_sb[i], wqk_d[i * P : (i + 1) * P, :], "wqk", "act")
        for i in range(3, KD):
            load(nc.gpsimd, wqk_sb[i], wqk_d[i * P : (i + 1) * P, :], "wqk", "act")
        for nb in range(1, NQB):
            for i in range(KD):
                load_xt_piece(i, nb)
        for i in range(KD):
            load(nc.sync, wv_sb[i], wv_d[i * P : (i + 1) * P, :], "wv", "dve")
        # output projection weights: h0+h1 stacked on all 128 partitions so
        # each proj chunk is one K=128 matmul plus a K=64 for h2
        wp01 = const.tile([P, D], BF16, tag="wp01")
        load(nc.gpsimd, wp01, wp_d[0 : 2 * HD, :], "wp01", "act")
        wp2 = const.tile([HD, D], BF16, tag="wp2")
        load(nc.gpsimd, wp2, wp_d[2 * HD : 3 * HD, :], "wp2", "act")

        # v1 tiles ([v_h | ones] per head) memset early, during the loads
        v1 = []
        for st in range(NKC):
            t = const.tile([P, HPC * (HD + 1)], BF16, tag=f"v1_{st}")
            nc.vector.memset(t[:], 1.0)
            v1.append(t)

        # ---- Q^T / K^T projection, k-outer in 2 passes of 8 psum banks ----
        # h2's Q/K are produced at BOTH partition bases so score pairs can
        # always put one lhsT at rows 0-63 and the other at rows 64-127.
        tq0 = const.tile([P, S], BF16, tag="tq0")  # [q_h0; q_h1]
        tk0 = const.tile([P, S], BF16, tag="tk0")  # [k_h0; k_h1]
        tq1 = const.tile([HD, S], BF16, tag="tq1")  # q_h2 @ base 0
        tk1 = const.tile([HD, S], BF16, tag="tk1")  # k_h2 @ base 0
        tq1h = const.tile([P, S], BF16, tag="tq1h")  # rows 64:128 = q_h2
        tk1h = const.tile([P, S], BF16, tag="tk1h")  # rows 64:128 = k_h2
        CQ2, CK2 = P, 2 * P + HD  # wqk col starts of q2 / k2
        # pass -> (full-m spec, colpair spec): colpair writes lo/hi halves of
        # one psum bank concurrently (auto tile_position (0,0) / (0,64)).
        passes = [
            ((tq0, 0, 0), (tq1, CQ2, 1, tk1h, CK2, 3)),
            ((tk0, P + HD, 2), (tk1, CK2, 3, tq1h, CQ2, 1)),
        ]
        qk_ctx = ExitStack()
        ps_qk = qk_ctx.enter_context(tc.tile_pool(name="ps_qk", bufs=8, space="PSUM"))
        for (dst0, c00, b0), (dlo, clo, blo, dhi, chi, bhi) in passes:
            # bank-outer / k-inner: each nb's psum banks finish early so the
            # bias-add drains overlap the next bank's matmuls instead of
            # piling up (PE-idle) at the end of the pass
            for nb in range(NQB):
                psf = ps_qk.tile([P, QB], F32, tag="qk", name=f"psf{b0}_{nb}")
                psc = ps_qk.tile([P, QB], F32, tag="qk", name=f"psc{b0}_{nb}")
                for k in range(KD):
                    x_ap = xt[k][:, nb * QB : (nb + 1) * QB]
                    st = dict(start=(k == 0), stop=(k == KD - 1))
                    nc.tensor.matmul(psf[:], wqk_sb[k][:, c00 : c00 + P], x_ap, **st)
                    nc.tensor.matmul(psc[0:HD], wqk_sb[k][:, clo : clo + HD], x_ap, **st)
                    nc.tensor.matmul(psc[HD:P], wqk_sb[k][:, chi : chi + HD], x_ap, **st)
                cols = slice(nb * QB, (nb + 1) * QB)
                # bias-adds drain the psum banks, balanced across ACT and DVE
                if nb % 2:
                    nc.vector.tensor_scalar_add(dst0[:, cols], psf[:], bqk_sb[:, b0 : b0 + 1])
                    nc.vector.tensor_scalar_add(dlo[:, cols], psc[0:HD], bqk_sb[0:HD, blo : blo + 1])
                    nc.scalar.activation(dhi[HD:P, cols], psc[HD:P],
                                         mybir.ActivationFunctionType.Identity,
                                         bias=bqk_sb[HD:P, bhi : bhi + 1])
                else:
                    nc.scalar.activation(dst0[:, cols], psf[:],
                                         mybir.ActivationFunctionType.Identity,
                                         bias=bqk_sb[:, b0 : b0 + 1])
                    nc.scalar.activation(dlo[:, cols], psc[0:HD],
                                         mybir.ActivationFunctionType.Identity,
                                         bias=bqk_sb[0:HD, blo : blo + 1])
                    nc.vector.tensor_scalar_add(dhi[HD:P, cols], psc[HD:P], bqk_sb[HD:P, bhi : bhi + 1])
        qk_ctx.close()

        # ---- V projection, k-outer in 2 passes of 8 key chunks ----
        v_ctx = ExitStack()
        ps_v = v_ctx.enter_context(tc.tile_pool(name="ps_v", bufs=8, space="PSUM"))
        for pas in (range(0, 8), range(8, NKC)):
            psl = [ps_v.tile([P, HPC * HD], F32, tag="v", name=f"pv{st}") for st in pas]
            # chunk-outer so each psum bank finishes (and its drain + the
            # next tenant's matmuls start) without waiting for the whole pass
            for st, pv in zip(pas, psl):
                for k in range(KD):
                    nc.tensor.matmul(
                        pv[:],
                        xt[k][:, st * P : (st + 1) * P],
                        wv_sb[k][:],
                        start=(k == 0),
                        stop=(k == KD - 1),
                    )
                # drain psum -> v1 head blocks, alternating DVE/ACT per chunk
                for h in range(HPC):
                    dst = v1[st][:, h * (HD + 1) : h * (HD + 1) + HD]
                    src = pv[:, h * HD : (h + 1) * HD]
                    if st % 2:
                        nc.scalar.copy(dst, src)
                    else:
                        nc.vector.tensor_copy(dst, src)
        v_ctx.close()

        # ---- attention steady state ----
        attn_ctx = ExitStack()
        ps_s = attn_ctx.enter_context(tc.tile_pool(name="ps_s", bufs=3, space="PSUM"))
        ps_o = attn_ctx.enter_context(tc.tile_pool(name="ps_o", bufs=4, space="PSUM"))
        ps_x = attn_ctx.enter_context(tc.tile_pool(name="ps_x", bufs=1, space="PSUM"))

        def qk_loc(h, base64):
            # -> (q tile, k tile, partition offset)
            if h == 0:
                return tq0, tk0, 0
            if h == 1:
                return tq0, tk0, HD
            return (tq1h, tk1h, HD) if base64 else (tq1, tk1, 0)

        units = [(qb, h) for qb in range(NQB) for h in range(HPC)]

        def emit_scores(pair, bases, g):
            # one key chunk per round: 2 score MMs co-run (rows split via the
            # 0/64 lhsT bases), then one exp per engine (ACT / DVE alternate
            # per round so neither engine saturates).
            ess = []
            for i, ((qb, h), b64) in enumerate(zip(pair, bases)):
                qt, kt, o = qk_loc(h, b64)
                ps = ps_s.tile([P, QB], F32, tag="s", name=f"ps{qb}_{h}_{g}")
                nc.tensor.matmul(
                    ps[:],
                    kt[o : o + HD, g * P : (g + 1) * P],
                    qt[o : o + HD, qb * QB : (qb + 1) * QB],
                    start=True,
                    stop=True,
                )
                es = es_pool.tile([P, QB], BF16, tag="es", name=f"es{qb}_{h}_{g}")
                if (g + i) % 2:
                    nc.vector.tensor_scalar(
                        out=es[:].bitcast(I16), in0=ps[:], scalar1=A16, scalar2=B16,
                        op0=MULT, op1=ADD,
                    )
                else:
                    nc.scalar.activation(es[:], ps[:], EXP)
                ess.append(es)
            return ess

        def emit_av(pair, pos, ess, kc):
            # one K=128 matmul per unit into a single-region po [65, 512]:
            # row 64 (the v1 ones column) accumulates the softmax denominator
            # in place, so normalize needs no halves-add.
            for (qb, h), po, es in zip(pair, pos, ess):
                nc.tensor.matmul(
                    po[:],
                    v1[kc][:, h * (HD + 1) : (h + 1) * (HD + 1)],
                    es[:],
                    start=(kc == 0),
                    stop=(kc == NKC - 1),
                )

        def normalize_a(u, po, i):
            # All the po (psum) readers, emitted at the pair boundary so the
            # po slot recycles before the next pair's AV needs it.  One
            # [1,512] bf16-converting denom-row copy + one [64,512] copy,
            # split across ACT/DVE (alternating by unit).
            dsb2h = rc_pool.tile([HD + 1, QB], BF16, tag="rc2")
            ocp = rc_pool.tile([HD, QB], F32, tag="ocp")
            if i % 2:
                nc.scalar.copy(dsb2h[HD : HD + 1, :], po[HD : HD + 1, :])
                nc.vector.tensor_copy(ocp[:], po[0:HD, :])
            else:
                nc.vector.tensor_copy(dsb2h[HD : HD + 1, :], po[HD : HD + 1, :])
                nc.scalar.copy(ocp[:], po[0:HD, :])
            return dsb2h, ocp

        ot01_map = {}  # qb -> stacked [h0; h1] ot tile

        def normalize_b(u, dsb2h, ocp):
            # broadcast the denom from row 64 to rows 0-63 on the PE (bf16
            # matmul — stays in the matmul stream, keeps HAM warm);
            # reciprocal must run at base partition 0 (custom DVE ops are
            # silent no-ops elsewhere).  h0/h1 outputs land stacked on
            # partitions 0-63 / 64-127 (h1 via an SBUF->SBUF DMA partition
            # shift, off the critical path) so each projection chunk is one
            # K=128 matmul + one K=64 instead of three K=64.
            qb, h = u
            prb = ps_x.tile([HD, QB], F32, tag="x", name=f"prb{qb}_{h}")
            nc.tensor.matmul(
                prb[:], ones_bf[HD : HD + 1, :], dsb2h[HD : HD + 1, :],
                start=True, stop=True,
            )
            rb = rc_pool.tile([HD, QB], F32, tag="rb")
            nc.vector.reciprocal_approx_fast(rb[:], prb[:])
            if h == 0:
                ot01 = ot_pool.tile([P, QB], BF16, tag="ot01", name=f"ot01_{qb}")
                ot01_map[qb] = [ot01]
                nc.gpsimd.tensor_tensor(out=ot01[0:HD, :], in0=ocp[:], in1=rb[:], op=MULT)
            elif h == 1:
                tmp = ot_pool.tile([HD, QB], BF16, tag="otmp", name=f"otmp_{qb}")
                nc.gpsimd.tensor_tensor(out=tmp[:], in0=ocp[:], in1=rb[:], op=MULT)
                nc.sync.dma_start(out=ot01_map[qb][0][HD:P, :], in_=tmp[:])
            else:
                ot2 = ot_pool.tile([HD, QB], BF16, tag="ot2", name=f"ot2_{qb}")
                nc.gpsimd.tensor_tensor(out=ot2[:], in0=ocp[:], in1=rb[:], op=MULT)
                pending_proj.append((qb, (*ot01_map.pop(qb), ot2), 0))

        pending_proj = []  # (qb, [ot_h0, ot_h1, ot_h2], next_dchunk)
        proj_state = [None]  # (qb, ots, dc, pp, next_h) mid-chunk
        tail_rot = [0]

        def emit_proj_step(tail=False):
            # one projection matmul per call; the psum drain (split across
            # ACT and DVE) + output DMA ride the call that finishes a chunk.
            # At the tail the scores/po psum slots are idle, so chunks rotate
            # through all three pools and the drains overlap the matmuls.
            if proj_state[0] is None:
                if not pending_proj:
                    return False
                qb, ots, dc = pending_proj[0]
                if dc + 1 == KD:
                    pending_proj.pop(0)
                else:
                    pending_proj[0] = (qb, ots, dc + 1)
                if tail:
                    pool, tag = [(ps_x, "x"), (ps_s, "s"), (ps_o, "o")][tail_rot[0] % 3]
                    tail_rot[0] += 1
                else:
                    pool, tag = ps_x, "x"
                pp = pool.tile([P, QB], F32, tag=tag, name=f"pp{qb}_{dc}")
                proj_state[0] = (qb, ots, dc, pp, 0)
            qb, ots, dc, pp, h = proj_state[0]
            cols = slice(dc * P, (dc + 1) * P)
            lhsT = [wp01[:, cols], wp2[:, cols]][h]
            rhs = ots[h][:, :]
            nc.tensor.matmul(pp[:], lhsT, rhs, start=(h == 0), stop=(h == 1))
            if h == 1:
                # bf16 partials: halves the output DMA; the host f32-sums the
                # 4 per-batch partials so the rounding stays ~0.2% per term
                outsb = out_pool.tile([P, QB], BF16, tag="outsb")
                nc.scalar.copy(outsb[:, 0 : QB // 2], pp[:, 0 : QB // 2])
                nc.vector.tensor_copy(outsb[:, QB // 2 :], pp[:, QB // 2 :])
                nc.gpsimd.dma_start(
                    out=out_d[dc * P : (dc + 1) * P, qb * QB : (qb + 1) * QB],
                    in_=outsb[:].bitcast(mybir.dt.uint16),
                )
                proj_state[0] = None
            else:
                proj_state[0] = (qb, ots, dc, pp, h + 1)
            return True

        pending_norm = []  # (u, dsb2h, ocp) after normalize_a
        for p0 in range(0, len(units), 2):
            pair = units[p0 : p0 + 2]
            last_pair = p0 + 2 >= len(units)
            # one unit's Q/K at partition base 0, the other at base 64
            bases = []
            seen64 = False
            for qb, h in pair:
                if h == 1:
                    bases.append(True)
                    seen64 = True
                else:
                    bases.append(False)
            if not seen64:  # (h2, h0) pair: put h2 at base 64
                bases = [h == 2 for qb, h in pair]
            # free the previous pair's po slots first: emit their psum reads
            # now (ACT/DVE run them while the PE streams this pair's first
            # scores / the previous pair's last AV).
            normed = [(u, *normalize_a(u, po, i)) for i, (u, po) in enumerate(pending_norm)]
            pending_norm = []
            pos = [
                ps_o.tile([HD + 1, QB], F32, tag="o", name=f"po_{p0}_{i}")
                for i in range(len(pair))
            ]
            es_hist = []
            for g in range(NKC):
                # AV lags the scores by TWO rounds: exp(g) gets ~2 rounds of
                # slack before its es is consumed, and the new pair's first
                # AV lands after the old pair's po slots have recycled.
                if g >= 2:
                    emit_av(pair, pos, es_hist[g - 2], g - 2)
                es_hist.append(emit_scores(pair, bases, g))
                # previous pair's normalize tails ride rounds 2-3; one or two
                # projection matmuls bridge the other rounds (two when the
                # backlog is deep, so it never piles up into a cold tail).
                if g in (2, 3) and normed:
                    normalize_b(*normed.pop(0))
                else:
                    # on the last pair, reserve ~3 chunks of projection work
                    # so the PE stays busy (and HAM stays warm) through the
                    # tail's normalize chains
                    rem = sum(KD - dc for _, _, dc in pending_proj)
                    if not (last_pair and rem <= 2):
                        emit_proj_step()
                        rem = sum(KD - dc for _, _, dc in pending_proj)
                        if rem >= 4 or (proj_state[0] is not None and rem >= 2):
                            emit_proj_step()
            emit_av(pair, pos, es_hist[NKC - 2], NKC - 2)
            emit_av(pair, pos, es_hist[NKC - 1], NKC - 1)
            for u, po in zip(pair, pos):
                pending_norm.append((u, po))
        # tail: the last pair's norms run on ACT/DVE/gpsimd while the PE
        # chews the reserved projection chunks; the final qb then projects
        # as two dense waves (all h01 matmuls across 6 idle psum banks,
        # then all h2 matmuls + drains) so the PE never idles long enough
        # to get clock-throttled.
        normed = [(u, *normalize_a(u, po, i)) for i, (u, po) in enumerate(pending_norm)]
        while proj_state[0] is not None or pending_proj:
            emit_proj_step(tail=True)
        for item in normed:
            normalize_b(*item)
        assert len(pending_proj) == 1 and pending_proj[0][2] == 0
        qb, ots, _ = pending_proj.pop(0)
        slots = [(ps_x, "x"), (ps_s, "s"), (ps_s, "s"), (ps_s, "s"), (ps_o, "o"), (ps_o, "o")]
        pps = []
        for dc in range(KD):
            pool, tag = slots[dc]
            pp = pool.tile([P, QB], F32, tag=tag, name=f"ppw{qb}_{dc}")
            nc.tensor.matmul(
                pp[:], wp01[:, dc * P : (dc + 1) * P], ots[0][:, :],
                start=True, stop=False,
            )
            pps.append(pp)
        for dc in range(KD):
            nc.tensor.matmul(
                pps[dc][:], wp2[:, dc * P : (dc + 1) * P], ots[1][:, :],
                start=False, stop=True,
            )
            outsb = out_pool.tile([P, QB], BF16, tag="outsb")
            nc.scalar.copy(outsb[:, 0 : QB // 2], pps[dc][:, 0 : QB // 2])
            nc.vector.tensor_copy(outsb[:, QB // 2 :], pps[dc][:, QB // 2 :])
            nc.gpsimd.dma_start(
                out=out_d[dc * P : (dc + 1) * P, qb * QB : (qb + 1) * QB],
                in_=outsb[:].bitcast(mybir.dt.uint16),
            )

        attn_ctx.close()

    nc.compile()
    return nc


def get_nc():
    if "nc" not in _CACHE:
        _CACHE["nc"] = _build_nc()
    return _CACHE["nc"]


def _bf16_bits(a):
    u = np.ascontiguousarray(a, np.float32).view(np.uint32)
    return ((u + 0x7FFF + ((u >> 16) & 1)) >> 16).astype(np.uint16)


def shard_inputs(x, w_qkv, b_qkv, w_proj):
    x = np.asarray(x, np.float32)
    w_qkv = np.asarray(w_qkv, np.float32)
    b_qkv = np.asarray(b_qkv, np.float32)
    w_proj = np.asarray(w_proj, np.float32)
    Wq, Wk = w_qkv[:, :D], w_qkv[:, D : 2 * D]
    Wv = w_qkv[:, 2 * D :]
    bq, bk = b_qkv[:D], b_qkv[D : 2 * D]
    in_maps = []
    for c in range(NCORES):
        b = c // 4
        lo = HD * HPC * (c % 4)
        sl = slice(lo, lo + HPC * HD)
        bq_s = bq[sl] * SCALE
        bk_s = bk[sl]
        bias4 = np.zeros((P, 4), np.float32)
        bias4[:, 0] = bq_s[0:P]
        bias4[:HD, 1] = bq_s[P : P + HD]
        bias4[HD:, 1] = bq_s[P : P + HD]
        bias4[:, 2] = bk_s[0:P]
        bias4[:HD, 3] = bk_s[P : P + HD]
        bias4[HD:, 3] = bk_s[P : P + HD]
        in_maps.append(
            {
                "xT": _bf16_bits(x[b].T),
                "wqk": _bf16_bits(
                    np.concatenate([Wq[:, sl] * SCALE, Wk[:, sl]], axis=1)
                ),
                "bqk": bias4,
                "wv": _bf16_bits(Wv[:, sl]),
                "wp": _bf16_bits(w_proj[sl, :]),
            }
        )
    return in_maps


def assemble(outs, w_qkv, b_qkv, w_proj, b_proj):
    b_proj = np.asarray(b_proj, np.float32)
    bv = np.asarray(b_qkv, np.float32)[2 * D :]
    # device drops the V bias; bv contributes bv @ w_proj to every row
    bias = b_proj + bv @ np.asarray(w_proj, np.float32)
    def to_f32(o):
        o = np.asarray(o)
        if o.dtype == np.uint16:  # bf16 bit patterns
            return (o.astype(np.uint32) << 16).view(np.float32)
        return o.astype(np.float32)

    y = np.empty((B, S, D), np.float32)
    for b in range(B):
        acc = to_f32(outs[4 * b])
        for i in range(1, 4):
            acc = acc + to_f32(outs[4 * b + i])
        y[b] = acc.T + bias
    return y


def run(inputs, trace=False, **kw):
    nc = get_nc()
    in_maps = shard_inputs(
        inputs["x"], inputs["w_qkv"], inputs["b_qkv"], inputs["w_proj"]
    )
    res = run_bass_kernel_spmd(
        nc, in_maps, core_ids=list(range(NCORES)), trace=trace, **kw
    )
    outs = [r["out"] for r in res.results]
    return (
        assemble(outs, inputs["w_qkv"], inputs["b_qkv"], inputs["w_proj"], inputs["b_proj"]),
        res,
    )


def kernel(x, w_qkv, b_qkv, w_proj, b_proj):
    y, _ = run(
        {"x": x, "w_qkv": w_qkv, "b_qkv": b_qkv, "w_proj": w_proj, "b_proj": b_proj}
    )
    return y
